# revision 4
# baseline (speedup 1.0000x reference)
"""Trainium2 Bass kernel for nn_DAGNet (gnn_message_passing).

Strategy: data-parallel over batch (16 -> 2 samples/core x 8 cores).
BatchNorm batch-stats via one small AllReduce per DAG level-group.
Nodes packed 4-per-128-partitions; convs = 9 full-array [128,128]x[128,512]
matmuls/chunk (block-diagonal weights); edge-weighted sums = one matmul per
(src-group, dst-group) pair using host-packed sparse masks scaled on-device
by the BN alpha. bf16 operands, fp32 PSUM accumulation.
"""
import numpy as np
import ml_dtypes

N_NODES = 32
EDGE_P = 0.15
IN_PLANES = 16
PLANES = 32
BATCH = 16
HW = 128
EPS = 1e-5
N_CORES = 8
SPC = BATCH // N_CORES          # samples per core = 2
OHW = HW // 2                   # 64
FREE = SPC * OHW * OHW          # 8192 packed pixels / partition
NCH = 16                        # chunks of 512
PADF = SPC * 66 * 66            # padded A free size = 8712
N_TOT = float(BATCH * OHW * OHW)  # BN count = 65536


def _graph():
    rng = np.random.RandomState(42)
    preds = [[] for _ in range(N_NODES)]
    for j in range(1, N_NODES):
        for i in range(j):
            if rng.rand() < EDGE_P:
                preds[j].append(i)
    has_succ = [False] * N_NODES
    for j in range(N_NODES):
        for i in preds[j]:
            has_succ[i] = True
    bottom = [n for n in range(N_NODES) if not has_succ[n]]
    return preds, bottom


PREDS, BOTTOM = _graph()
INPUT_NODES = [n for n in range(N_NODES) if not PREDS[n]]
INNER_NODES = [n for n in range(N_NODES) if PREDS[n]]
IN_IDX = {n: i for i, n in enumerate(INPUT_NODES)}
INNER_IDX = {n: i for i, n in enumerate(INNER_NODES)}

# level groups: 0,1 = input conv groups; 2..8 inner. All preds of a node lie
# in strictly earlier groups (verified).
GROUPS = [[0, 1, 2, 3], [6, 10], [4, 5, 7, 11], [8, 9, 12, 15],
          [13, 16, 18, 21], [14, 19, 22, 23], [17, 20, 24, 26],
          [25, 27, 28, 31], [29, 30]]
NG = len(GROUPS)
NODE_GI = {}
NODE_COL = {}
for gi, g in enumerate(GROUPS):
    for c, n in enumerate(g):
        NODE_GI[n] = gi
        NODE_COL[n] = c
GBASE = [64 if gi == 8 else 0 for gi in range(NG)]   # G7 lives at partitions 64:128
GP = [32 * len(g) for g in GROUPS]                    # partition count per group

# (src_gi -> dst) pairs for weighted sums; dst 'F' = final mean of bottoms
PAIRS = {}
for gi in range(2, NG):
    s = sorted({NODE_GI[p] for q in GROUPS[gi] for p in PREDS[q]})
    PAIRS[gi] = s
PAIRS['F'] = sorted({NODE_GI[b] for b in BOTTOM})
DSTS = list(range(2, NG)) + ['F']


def _mask_layout():
    off = {}
    o = 0
    for d in DSTS:
        pd = 32 if d == 'F' else GP[d]
        for s in PAIRS[d]:
            off[(s, d)] = o
            o += pd
    return off, o


MOFF, MTOT = _mask_layout()


def _pack_weights(conv_in, conv_inner, edge_w, gamma, beta):
    bf = ml_dtypes.bfloat16
    sig = 1.0 / (1.0 + np.exp(-edge_w.astype(np.float64)))
    # inner conv block-diagonal stationaries: [128, sum(9*GP)]
    wcv = np.zeros((128, sum(9 * GP[gi] for gi in range(2, NG))), np.float32)
    woff = {}
    o = 0
    for gi in range(2, NG):
        woff[gi] = o
        pw = GP[gi]
        b = GBASE[gi]
        for c, n in enumerate(GROUPS[gi]):
            W = conv_inner[INNER_IDX[n]]          # [co, ci, 3, 3]
            for dy in range(3):
                for dx in range(3):
                    t = dy * 3 + dx
                    wcv[b + 32 * c:b + 32 * c + 32,
                        o + t * pw + 32 * c: o + t * pw + 32 * c + 32] = W[:, :, dy, dx].T
        o += 9 * pw
    # input conv stationaries [48, 3*128] and [48, 3*64]
    wia = np.zeros((48, 3 * 128), np.float32)
    wib = np.zeros((48, 3 * 64), np.float32)
    for c, n in enumerate(GROUPS[0]):
        W = conv_in[IN_IDX[n]]                    # [co, ci, 3, 3]
        for k in range(3):
            for dx in range(3):
                wia[16 * k:16 * k + 16, dx * 128 + 32 * c: dx * 128 + 32 * c + 32] = W[:, :, k, dx].T
    for c, n in enumerate(GROUPS[1]):
        W = conv_in[IN_IDX[n]]
        for k in range(3):
            for dx in range(3):
                wib[16 * k:16 * k + 16, dx * 64 + 32 * c: dx * 64 + 32 * c + 32] = W[:, :, k, dx].T
    # edge masks
    wm = np.zeros((128, MTOT), np.float32)
    for d in DSTS:
        pd = 32 if d == 'F' else GP[d]
        for s in PAIRS[d]:
            o = MOFF[(s, d)]
            if d == 'F':
                for b_ in BOTTOM:
                    if NODE_GI[b_] != s:
                        continue
                    r = GBASE[s] + 32 * NODE_COL[b_]
                    wm[r:r + 32, o:o + 32] += 0.25 * np.eye(32, dtype=np.float32)
            else:
                for q in GROUPS[d]:
                    for pos, p in enumerate(PREDS[q]):
                        if NODE_GI[p] != s:
                            continue
                        r = GBASE[s] + 32 * NODE_COL[p]
                        cc = 32 * NODE_COL[q]
                        wm[r:r + 32, o + cc:o + cc + 32] += sig[q, pos] * np.eye(32, dtype=np.float32)
    # gamma/beta per group slot [128, 2*NG]
    gb = np.zeros((128, 2 * NG), np.float32)
    for gi, g in enumerate(GROUPS):
        for c, n in enumerate(g):
            r = GBASE[gi] + 32 * c
            gb[r:r + 32, 2 * gi] = gamma[n]
            gb[r:r + 32, 2 * gi + 1] = beta[n]
    return (wcv.astype(bf), wia.astype(bf), wib.astype(bf), wm.astype(bf), gb, woff)


def _build(woff):
    from concourse import bass, bacc, tile, mybir
    dt = mybir.dt
    AF = mybir.ActivationFunctionType

    nc = bacc.Bacc("TRN2", target_bir_lowering=False, debug=False,
                   num_devices=N_CORES)
    x_t = nc.dram_tensor("x", [SPC, IN_PLANES, HW, HW], dt.float32, kind="ExternalInput")
    wcv_t = nc.dram_tensor("wcv", [128, sum(9 * GP[gi] for gi in range(2, NG))], dt.bfloat16, kind="ExternalInput")
    wia_t = nc.dram_tensor("wia", [48, 384], dt.bfloat16, kind="ExternalInput")
    wib_t = nc.dram_tensor("wib", [48, 192], dt.bfloat16, kind="ExternalInput")
    wm_t = nc.dram_tensor("wm", [128, MTOT], dt.bfloat16, kind="ExternalInput")
    gb_t = nc.dram_tensor("gb", [128, 2 * NG], dt.float32, kind="ExternalInput")
    out_t = nc.dram_tensor("out", [SPC, PLANES, OHW, OHW], dt.float32, kind="ExternalOutput")

    RG = [list(range(N_CORES))]

    with tile.TileContext(nc) as tc:
        with tc.tile_pool(name="zp", bufs=1) as zp, \
             tc.tile_pool(name="ap", bufs=3) as apool, \
             tc.tile_pool(name="wp", bufs=2) as wpool, \
             tc.tile_pool(name="sp", bufs=2) as spool, \
             tc.tile_pool(name="tp", bufs=4) as tpool, \
             tc.tile_pool(name="pc", bufs=2, space="PSUM") as pcp, \
             tc.tile_pool(name="ph", bufs=2, space="PSUM") as php, \
             tc.tile_pool(name="pb", bufs=2, space="PSUM") as pbp, \
             tc.tile_pool(name="dr", bufs=2, space="DRAM") as drp:

            # persistent tensors
            Z = {}
            for gi in [0, 2, 3, 4, 5, 6, 7]:
                Z[gi] = zp.tile([128, FREE], dt.bfloat16, tag=f"Z{gi}", name=f"Z{gi}")
            Zsm = zp.tile([128, FREE], dt.bfloat16, tag="Zsm")  # [0:64]=G0b, [64:128]=G7
            alpha = zp.tile([128, NG], dt.float32, tag="alpha")
            bp32 = zp.tile([128, NG], dt.float32, tag="bp32")
            bp16 = zp.tile([128, NG], dt.bfloat16, tag="bp16")
            biasb = zp.tile([128, NG + 1], dt.float32, tag="biasb")  # h-evict biases; col NG = final
            gbs = zp.tile([128, 2 * NG], dt.float32, tag="gbs")
            wias = zp.tile([48, 384], dt.bfloat16, tag="wias")
            wibs = zp.tile([48, 192], dt.bfloat16, tag="wibs")
            arin = zp.tile([128, 4], dt.float32, tag="arin")
            epsb = zp.tile([128, 1], dt.float32, tag="epsb")
            nc.vector.memset(epsb[:], EPS)
            arout = zp.tile([128, 4], dt.float32, tag="arout")

            nc.sync.dma_start(gbs[:], gb_t.ap())
            nc.sync.dma_start(wias[:], wia_t.ap())
            nc.sync.dma_start(wibs[:], wib_t.ap())

            def zsl(gi, ch):
                """Z slice for group gi, chunk ch -> (ap, base, P)"""
                if gi == 1:
                    return Zsm[0:64, ch * 512:(ch + 1) * 512], 0, 64
                if gi == 8:
                    return Zsm[64:128, ch * 512:(ch + 1) * 512], 64, 64
                return Z[gi][0:GP[gi], ch * 512:(ch + 1) * 512], 0, GP[gi]

            # ---------------- input stage ----------------
            XS2 = apool.tile([128, SPC * 16 * HW], dt.float32, tag="A")
            xv = x_t.ap().rearrange("s ci (rb r8) c -> rb ci s r8 c", rb=8, r8=16)
            for rb in range(8):
                nc.sync.dma_start(
                    XS2[rb * 16:(rb + 1) * 16, :].rearrange("p (s r8 c) -> p s r8 c", s=SPC, r8=16),
                    xv[rb])
            RX2 = apool.tile([128, SPC * 16 * HW], dt.bfloat16, tag="A")
            nc.vector.tensor_scalar_max(RX2[:], XS2[:], 0.0)
            RX5 = RX2[:].rearrange("p (s r4 e c) -> p s r4 e c", s=SPC, r4=8, e=2)

            AinC = []
            for s in range(SPC):
                t = apool.tile([48, 64 * 130], dt.bfloat16, tag="A", name=f"AinC{s}")
                nc.vector.memset(t[:], 0.0)
                AinC.append(t)
            # replicate+shift ReLU(x) into dy-stacked stride-2 layout
            for s in range(SPC):
                a3 = AinC[s][:].rearrange("p (r c) -> p r c", r=64, c=130)
                for k in range(3):
                    for rb in range(8):
                        if k == 1:
                            src = RX5[rb * 16:(rb + 1) * 16, s, 0:8, 0, :]
                            dst = a3[16 * k:16 * k + 16, rb * 8:rb * 8 + 8, 1:129]
                        elif k == 2:
                            src = RX5[rb * 16:(rb + 1) * 16, s, 0:8, 1, :]
                            dst = a3[16 * k:16 * k + 16, rb * 8:rb * 8 + 8, 1:129]
                        else:  # k=0: dst row = rb*8 + r4 + 1, clip at 63
                            nr = 7 if rb == 7 else 8
                            src = RX5[rb * 16:(rb + 1) * 16, s, 0:nr, 1, :]
                            dst = a3[16 * k:16 * k + 16, rb * 8 + 1:rb * 8 + 1 + nr, 1:129]
                        nc.sync.dma_start(dst, src)

            st6 = tpool.tile([128, 6 * NCH], dt.float32, tag="st")
            st6b = tpool.tile([128, 6 * NCH], dt.float32, tag="st")
            # input convs (G0a -> Z[0], G0b -> Zsm[0:64])
            for ch in range(NCH):
                s, rr = ch // 8, ch % 8
                a4 = AinC[s][:].rearrange("p (r cc u) -> p r cc u", r=64, cc=65, u=2)
                pa = pcp.tile([128, 512], dt.float32, tag="pc")
                pb_ = pcp.tile([64, 512], dt.float32, tag="pc2")
                for dx in range(3):
                    cc0, u0 = (0, 0) if dx == 0 else ((0, 1) if dx == 1 else (1, 0))
                    mv_ap = a4[0:48, 8 * rr:8 * rr + 8, cc0:cc0 + 64, u0:u0 + 1]
                    nc.tensor.matmul(pa[:, :], wias[0:48, dx * 128:(dx + 1) * 128], mv_ap,
                                     start=(dx == 0), stop=(dx == 2))
                for dx in range(3):
                    cc0, u0 = (0, 0) if dx == 0 else ((0, 1) if dx == 1 else (1, 0))
                    mv_ap = a4[0:48, 8 * rr:8 * rr + 8, cc0:cc0 + 64, u0:u0 + 1]
                    nc.tensor.matmul(pb_[:, :], wibs[0:48, dx * 64:(dx + 1) * 64], mv_ap,
                                     start=(dx == 0), stop=(dx == 2))
                nc.scalar.copy(Z[0][:, ch * 512:(ch + 1) * 512], pa[:, :])
                nc.scalar.copy(Zsm[0:64, ch * 512:(ch + 1) * 512], pb_[:, :])
                nc.vector.bn_stats(st6[:, ch * 6:(ch + 1) * 6], pa[:, :])
                nc.vector.bn_stats(st6b[0:64, ch * 6:(ch + 1) * 6], pb_[:, :])

            tmp = zp.tile([128, 8], dt.float32, tag="tmp")

            def stats_to_ar(st, b, P, col):
                mvt = tmp[b:b + P, 0:2]
                nc.vector.bn_aggr(mvt, st.rearrange("p (n k) -> p n k", n=NCH)[b:b + P])
                mean = mvt[:, 0:1]
                var = mvt[:, 1:2]
                sq = tmp[b:b + P, 2:3]
                nc.vector.tensor_mul(sq, mean, mean)
                s2 = tmp[b:b + P, 3:4]
                nc.vector.tensor_add(s2, var, sq)
                nc.vector.tensor_scalar_mul(arin[b:b + P, col:col + 1], mean, float(FREE))
                nc.vector.tensor_scalar_mul(arin[b:b + P, col + 1:col + 2], s2, float(FREE))

            def post_ar(gi, col):
                b, P = GBASE[gi], GP[gi]
                mu = tmp[b:b + P, 4:5]
                nc.vector.tensor_scalar_mul(mu, arout[b:b + P, col:col + 1], 1.0 / N_TOT)
                ex2 = tmp[b:b + P, 5:6]
                nc.vector.tensor_scalar_mul(ex2, arout[b:b + P, col + 1:col + 2], 1.0 / N_TOT)
                musq = tmp[b:b + P, 6:7]
                nc.vector.tensor_mul(musq, mu, mu)
                var = tmp[b:b + P, 7:8]
                nc.vector.tensor_sub(var, ex2, musq)
                sig = tmp[b:b + P, 2:3]
                nc.scalar.activation(sig, var, AF.Sqrt, bias=epsb[b:b + P, 0:1])
                rinv = tmp[b:b + P, 3:4]
                nc.vector.reciprocal(rinv, sig)
                al = alpha[b:b + P, gi:gi + 1]
                nc.vector.tensor_mul(al, gbs[b:b + P, 2 * gi:2 * gi + 1], rinv)
                t2 = tmp[b:b + P, 6:7]
                nc.vector.tensor_mul(t2, al, mu)
                bpr = bp32[b:b + P, gi:gi + 1]
                nc.vector.tensor_sub(bpr, gbs[b:b + P, 2 * gi + 1:2 * gi + 2], t2)
                nc.vector.tensor_copy(bp16[b:b + P, gi:gi + 1], bpr)

            def do_ar(sets):
                """sets: list of (st_tile, base, P, gi, col)"""
                for st, b, P, gi, col in sets:
                    stats_to_ar(st, b, P, col)
                din = drp.tile([128, 4], dt.float32, tag="din")
                dout = drp.tile([128, 4], dt.float32, tag="dout")
                nc.gpsimd.dma_start(din[:], arin[:])
                nc.gpsimd.collective_compute(
                    "AllReduce", mybir.AluOpType.add, replica_groups=RG,
                    ins=[din.opt()], outs=[dout.opt()])
                nc.sync.dma_start(arout[:], dout[:])
                for st, b, P, gi, col in sets:
                    post_ar(gi, col)

            do_ar([(st6, 0, 128, 0, 0), (st6b, 0, 64, 1, 2)])

            # ---------------- inner groups ----------------
            A = {}
            for di, d in enumerate(DSTS):
                pd = 32 if d == 'F' else GP[d]
                db = 0 if d == 'F' else GBASE[d]
                srcs = PAIRS[d]
                nsp = len(srcs)
                # masks for this dst
                wmt = wpool.tile([128, 768], dt.bfloat16, tag="wm")
                o0 = MOFF[(srcs[0], d)]
                nc.sync.dma_start(wmt[:, 0:nsp * pd], wm_t.ap()[:, o0:o0 + nsp * pd])
                # bias matmul: bias_d = sum_s mask_s^T @ bprime_s
                pbias = pbp.tile([128, 1], dt.float32, tag="pb")
                for j, s in enumerate(srcs):
                    sb = GBASE[s]
                    nc.tensor.matmul(pbias[db:db + pd, 0:1],
                                     wmt[sb:sb + GP[s], j * pd:(j + 1) * pd],
                                     bp16[sb:sb + GP[s], s:s + 1],
                                     start=(j == 0), stop=(j == nsp - 1))
                bcol = NG if d == 'F' else d
                nc.vector.tensor_copy(biasb[db:db + pd, bcol:bcol + 1], pbias[db:db + pd, 0:1])
                # alpha-scaled masks
                St = spool.tile([128, 768], dt.bfloat16, tag="S")
                for j, s in enumerate(srcs):
                    sb = GBASE[s]
                    nc.vector.tensor_scalar_mul(
                        St[sb:sb + GP[s], j * pd:(j + 1) * pd],
                        wmt[sb:sb + GP[s], j * pd:(j + 1) * pd],
                        alpha[sb:sb + GP[s], s:s + 1])

                if d == 'F':
                    # final: mean of bottoms -> out
                    ov = out_t.ap().rearrange("s co r c -> co s r c")
                    for ch in range(NCH):
                        s_, rr = ch // 8, ch % 8
                        pf = php.tile([128, 512], dt.float32, tag="ph")
                        for j, s in enumerate(srcs):
                            zr, sb, sP = zsl(s, ch)
                            nc.tensor.matmul(pf[0:32, :], St[GBASE[s]:GBASE[s] + GP[s], j * 32:(j + 1) * 32],
                                             zr, start=(j == 0), stop=(j == nsp - 1))
                        ot = tpool.tile([32, 512], dt.float32, tag="ot")
                        nc.vector.tensor_scalar_add(ot[:], pf[0:32, :], biasb[0:32, NG:NG + 1])
                        nc.sync.dma_start(
                            ov[0:32, s_:s_ + 1, 8 * rr:8 * rr + 8, :],
                            ot[:].rearrange("p (a r c) -> p a r c", a=1, r=8))
                    continue

                # weighted sum -> ReLU -> A_d
                Ad = apool.tile([128, PADF], dt.bfloat16, tag="A")
                nc.vector.memset(Ad[:], 0.0)
                a4 = Ad[:].rearrange("p (s r c) -> p s r c", s=SPC, r=66, c=66)
                for ch in range(NCH):
                    s_, rr = ch // 8, ch % 8
                    ph_ = php.tile([128, 512], dt.float32, tag="ph")
                    for j, s in enumerate(srcs):
                        zr, sb, sP = zsl(s, ch)
                        nc.tensor.matmul(ph_[db:db + pd, :],
                                         St[GBASE[s]:GBASE[s] + GP[s], j * pd:(j + 1) * pd],
                                         zr, start=(j == 0), stop=(j == nsp - 1))
                    nc.scalar.activation(
                        a4[db:db + pd, s_:s_ + 1, 8 * rr + 1:8 * rr + 9, 1:65],
                        ph_[db:db + pd, :], AF.Relu,
                        bias=biasb[db:db + pd, d:d + 1])

                # conv of this group
                wct = wpool.tile([128, 9 * 128], dt.bfloat16, tag="wc")
                nc.sync.dma_start(wct[db:db + pd, 0:9 * pd],
                                  wcv_t.ap()[db:db + pd, woff[d]:woff[d] + 9 * pd])
                stt = tpool.tile([128, 6 * NCH], dt.float32, tag="st")
                for ch in range(NCH):
                    s_, rr = ch // 8, ch % 8
                    pcv = pcp.tile([128, 512], dt.float32, tag="pc")
                    for dy in range(3):
                        for dx in range(3):
                            t = dy * 3 + dx
                            mv_ap = a4[db:db + pd, s_:s_ + 1, 8 * rr + dy:8 * rr + dy + 8, dx:dx + 64]
                            nc.tensor.matmul(pcv[db:db + pd, :],
                                             wct[db:db + pd, t * pd:(t + 1) * pd],
                                             mv_ap, start=(t == 0), stop=(t == 8))
                    zw, zb, zP = zsl(d, ch)
                    nc.scalar.copy(zw, pcv[db:db + pd, :])
                    nc.vector.bn_stats(stt[db:db + pd, ch * 6:(ch + 1) * 6], pcv[db:db + pd, :])
                do_ar([(stt, db, pd, d, 0)])

    nc.compile()
    return nc


_CACHED = {}


def kernel(x, conv_in, conv_inner, gamma, beta, edge_w):
    from concourse import bass_utils
    x = np.asarray(x, np.float32)
    wcv, wia, wib, wm, gb, woff = _pack_weights(
        np.asarray(conv_in, np.float32), np.asarray(conv_inner, np.float32),
        np.asarray(edge_w, np.float32), np.asarray(gamma, np.float32),
        np.asarray(beta, np.float32))
    if 'nc' not in _CACHED:
        _CACHED['nc'] = _build(woff)
    nc = _CACHED['nc']
    in_maps = []
    for c in range(N_CORES):
        in_maps.append({
            "x": np.ascontiguousarray(x[c * SPC:(c + 1) * SPC]),
            "wcv": wcv, "wia": wia, "wib": wib, "wm": wm, "gb": gb,
        })
    res = bass_utils.run_bass_kernel_spmd(nc, in_maps, core_ids=list(range(N_CORES)))
    _CACHED['last_res'] = res
    out = np.concatenate([res.results[c]["out"] for c in range(N_CORES)], axis=0)
    return out.astype(np.float32)
